# revision 27
# baseline (speedup 1.0000x reference)
"""Adaptive bilateral filter (nn_AdaptiveFilter) on 8 TRN2 NeuronCores.

Math: out_c(p) = sum_k x_c(p+d_k) * wt_k(p) / sum_k wt_k(p)
with wt_k = softmax_k(w)(p) * exp(-50 * (sum_c |g_c(p+d_k) - g_c(p)|)^2).
Softmax normalization cancels in num/den, so wt_k = E[src(k)] * exp(-50*s^2)
with E = exp(w0) precomputed on HOST (slot-major bf16) and src = reflect
map (7,7)->(4,4).

Sharding: 8 cores = 2 batches x 4 row-bands of 128 rows. Host reflect-pads
to (518,518), converts to bf16 and interleaves channels per row, shipping
each core a (134, 3*518) band (halo included) of g and x, E [128, 16*512]
bf16 slot-major, and the center tile gc [128, 3*512] bf16. No collectives.
Row-interleaved channels make every DMA descriptor a 3108-byte contiguous
run (vs 1036 for planar) and need no kernel-side AP changes.

Engine split per tap-row i (j-packed over 7 column taps, c-packed over 3
channels):
  DVE:    ONE bf16 subtract [128, 3*7*512] (sliding-window in0 vs broadcast
          center in1), wt = col*E (+-512-stride E views), ONE product
          x*wt [128, 3*7*512]
  ACT:    in-place Abs on the subtract output, Derivative_Erf(sqrt(50)*s)
          = 2/sqrt(pi)*exp(-50 s^2) from 2-bank PSUM pairs (the 2/sqrt(pi)
          cancels between num and den)
  PE:     channel-sum of |d| into PSUM (identity matmuls), den/num
          accumulation over the 49 taps
  DMA:    g on the sync queue, x + E chunks on the scalar queue; E_t is
          issued in slot t (not up-front) so it never starves g0/x0.
Emission is software-pipelined: slot k runs sub_k | DErf_{k-1} | abs_k |
wt/prod/den/num_{k-2}. Output is one packed bf16 [128, 3*512] DMA
(host casts up and de-interleaves).
"""
import sys
sys.path.insert(0, "/opt/trn_rl_repo")
import math
import numpy as np

import concourse.bacc as bacc
import concourse.mybir as mybir
import concourse.tile as tile
from concourse.ap import AP
from concourse.bass_utils import run_bass_kernel_spmd

F32 = mybir.dt.float32
U16 = mybir.dt.uint16
BF16 = mybir.dt.bfloat16
AF = mybir.ActivationFunctionType
OP = mybir.AluOpType

KH = KW = 7
H_BAND = 128
W = 512
WP = 518
WJ = KW * W        # 3584
CJ = 3 * WJ        # 10752
SCALE = math.sqrt(50.0)  # Square(sqrt(50)*s) = 50*s^2
PAIRS = ((0, 2), (2, 2), (4, 2), (6, 1))

_CACHE = {}


def _view(ap_obj, dims):
    """AP with the tile's partition dim plus the given free [stride, size]."""
    base = ap_obj.ap
    return AP(tensor=ap_obj.tensor, offset=ap_obj.offset,
              ap=[list(base[0])] + [list(d) for d in dims])


def _emit(nc, tc, constp, gxp, workp, finp, psump, g_d, x_d, e_d, gc_d,
          id_d, out_d):
    ident = constp.tile([128, 128], BF16, tag="ident", name="ident")
    nc.sync.dma_start(ident[:], id_d.ap()[:, :])

    gc = constp.tile([H_BAND, 3 * W], BF16, tag="gc", name="gc")
    nc.sync.dma_start(gc[:], gc_d.ap()[:, :])

    E = [constp.tile([H_BAND, 4 * W], BF16, tag=f"E{t}", name=f"E{t}")
         for t in range(4)]

    den_ps = psump.tile([H_BAND, W], F32, tag="dps", name="dps", bufs=1)
    num_wide = psump.tile([H_BAND, 3 * W], F32, tag="npsw", name="npsw",
                          bufs=1)

    stageA = {}
    stageB = {}

    def emit_A1(i):
        gt = gxp.tile([H_BAND, 3 * WP], BF16, tag="gt", name="gt", bufs=2)
        nc.sync.dma_start(gt[:], g_d.ap()[i:i + H_BAND, :])
        if i < 4:
            nc.sync.dma_start(E[i][:],
                              e_d.ap()[:, i * 4 * W:(i + 1) * 4 * W])
        # u[p, c, j, w] = gt[p, c*518 + j + w] - gc[p, c*512 + w]
        u = workp.tile([H_BAND, CJ], BF16, tag="u", name="u", bufs=2)
        nc.vector.tensor_tensor(
            u[:].rearrange("p (c n w) -> p c n w", c=3, n=KW),
            _view(gt[:], [[WP, 3], [1, KW], [1, W]]),
            _view(gc[:], [[W, 3], [0, KW], [1, W]]),
            OP.subtract)
        stageA[i] = (u, None)

    def emit_A2(i):
        u, _ = stageA[i]
        nc.scalar.activation(u[:], u[:], AF.Abs)

    def emit_B(i):
        u, _ = stageA.pop(i)
        # x_i isn't read until stage C (next slot): issuing its DMA here
        # keeps the early sync queue clear for g/E
        xt = gxp.tile([H_BAND, 3 * WP], BF16, tag="xt", name="xt", bufs=3)
        nc.sync.dma_start(xt[:], x_d.ap()[i:i + H_BAND, :])
        col = workp.tile([H_BAND, WJ], BF16, tag="col", name="col", bufs=2)
        for j in range(KW):
            s_ps = psump.tile([H_BAND, W], F32, tag="sps", name="sps",
                              bufs=4)
            for c in range(3):
                nc.tensor.matmul(
                    s_ps[:], ident[:],
                    u[:, c * WJ + j * W:c * WJ + (j + 1) * W],
                    start=(c == 0), stop=(c == 2))
            nc.scalar.activation(col[:, j * W:(j + 1) * W], s_ps[:],
                                 AF.Derivative_Erf, scale=SCALE)
        stageB[i] = (col, xt)

    TB = 516          # T block stride (515 valid + 1 pad)
    T3C = 3 * TB      # per-channel span of the 3 delta blocks in u3

    def emit_A1_m3():
        gt = gxp.tile([H_BAND, 3 * WP], BF16, tag="gt", name="gt", bufs=2)
        nc.sync.dma_start(gt[:], g_d.ap()[3:3 + H_BAND, :])
        nc.sync.dma_start(E[3][:], e_d.ap()[:, 12 * W:16 * W])
        # u3[p, c, b, a] = g3[p, c*518 + a + (3-b)] - g3[p, c*518 + a]
        u3 = workp.tile([H_BAND, 3 * T3C], BF16, tag="u", name="u", bufs=2)
        nc.vector.tensor_tensor(
            _view(u3[:], [[T3C, 3], [TB, 3], [1, 515]]),
            AP(tensor=gt[:].tensor, offset=gt[:].offset + 3,
               ap=[list(gt[:].ap[0]), [WP, 3], [-1, 3], [1, 515]]),
            _view(gt[:], [[WP, 3], [0, 3], [1, 515]]),
            OP.subtract)
        stageA[3] = (u3, None)

    def emit_A2_m3():
        u3, _ = stageA[3]
        nc.scalar.activation(u3[:], u3[:], AF.Abs)

    def emit_B_m3():
        u3, _ = stageA.pop(3)
        xt = gxp.tile([H_BAND, 3 * WP], BF16, tag="xt", name="xt", bufs=3)
        nc.sync.dma_start(xt[:], x_d.ap()[3:3 + H_BAND, :])
        # T: 3 blocks of 516 (D_3, D_2, D_1), each valid [0, 515)
        colT = workp.tile([H_BAND, T3C], BF16, tag="col", name="col", bufs=2)
        for b in range(3):
            sa = psump.tile([H_BAND, W], F32, tag="sps", name="sps", bufs=4)
            sb = psump.tile([H_BAND, W], F32, tag="sps", name="sps", bufs=4)
            for c in range(3):
                nc.tensor.matmul(sa[:], ident[:],
                                 u3[:, c * T3C + b * TB:c * T3C + b * TB + W],
                                 start=(c == 0), stop=(c == 2))
            for c in range(3):
                nc.tensor.matmul(
                    sb[:, 0:3], ident[:],
                    u3[:, c * T3C + b * TB + W:c * T3C + b * TB + 515],
                    start=(c == 0), stop=(c == 2))
            nc.scalar.activation(colT[:, b * TB:b * TB + W], sa[:],
                                 AF.Derivative_Erf, scale=SCALE)
            nc.scalar.activation(colT[:, b * TB + W:b * TB + 515],
                                 sb[:, 0:3], AF.Derivative_Erf, scale=SCALE)
        stageB[3] = (colT, xt)

    def emit_C_m3():
        colT, xt = stageB.pop(3)
        eb = E[3][:]
        wt = workp.tile([H_BAND, WJ], BF16, tag="wt", name="wt", bufs=2)
        # j=0..2: col_j(w) = T[block 3-j.., w+j] -> offset j*(TB+1)
        nc.vector.tensor_tensor(
            wt[:, 0:3 * W].rearrange("p (n w) -> p n w", n=3),
            AP(tensor=colT[:].tensor, offset=colT[:].offset,
               ap=[list(colT[:].ap[0]), [TB + 1, 3], [1, W]]),
            _view(eb, [[W, 3], [1, W]]), OP.mult)
        # j=3: wt = E slot 3 (D_0 == 1)
        nc.vector.tensor_copy(wt[:, 3 * W:4 * W], eb[:, 3 * W:4 * W])
        # j=4..6: col_j(w) = T[block j-3, w+3] -> offset (6-j)*TB+3
        nc.vector.tensor_tensor(
            wt[:, 4 * W:].rearrange("p (n w) -> p n w", n=3),
            AP(tensor=colT[:].tensor, offset=colT[:].offset + 2 * TB + 3,
               ap=[list(colT[:].ap[0]), [-TB, 3], [1, W]]),
            AP(tensor=eb.tensor, offset=eb.offset + 2 * W,
               ap=[list(eb.ap[0]), [-W, 3], [1, W]]), OP.mult)
        for j in range(KW):
            nc.tensor.matmul(den_ps[:], ident[:], wt[:, j * W:(j + 1) * W],
                             start=False, stop=False)
        prod = workp.tile([H_BAND, CJ], BF16, tag="pr", name="pr", bufs=2)
        nc.vector.tensor_tensor(
            prod[:].rearrange("p (c n w) -> p c n w", c=3, n=KW),
            _view(xt[:], [[WP, 3], [1, KW], [1, W]]),
            _view(wt[:], [[0, 3], [W, KW], [1, W]]),
            OP.mult)
        for c in range(3):
            for j in range(KW):
                nc.tensor.matmul(
                    num_wide[:, c * W:(c + 1) * W], ident[:],
                    prod[:, c * WJ + j * W:c * WJ + (j + 1) * W],
                    start=False, stop=False)

    def emit_C(i):
        col, xt = stageB.pop(i)
        ri = min(i, 6 - i)
        first_i, last_i = (i == 0), (i == 6)
        eb = E[ri][:]
        # wt = col * E(src tap): j in 0..3 reads E[ri] slots 0..3 (+W step),
        # j in 4..6 reads slots 2..0 (-W step)
        wt = workp.tile([H_BAND, WJ], BF16, tag="wt", name="wt", bufs=2)
        nc.vector.tensor_tensor(
            wt[:, 0:4 * W].rearrange("p (n w) -> p n w", n=4),
            col[:, 0:4 * W].rearrange("p (n w) -> p n w", n=4),
            _view(eb, [[W, 4], [1, W]]), OP.mult)
        nc.vector.tensor_tensor(
            wt[:, 4 * W:].rearrange("p (n w) -> p n w", n=3),
            col[:, 4 * W:].rearrange("p (n w) -> p n w", n=3),
            AP(tensor=eb.tensor, offset=eb.offset + 2 * W,
               ap=[list(eb.ap[0]), [-W, 3], [1, W]]), OP.mult)
        for j in range(KW):
            nc.tensor.matmul(den_ps[:], ident[:], wt[:, j * W:(j + 1) * W],
                             start=(first_i and j == 0),
                             stop=(last_i and j == 6))
        # prod[p, c, j, w] = xt[p, c*518 + j + w] * wt[p, j*512 + w]
        prod = workp.tile([H_BAND, CJ], BF16, tag="pr", name="pr", bufs=2)
        nc.vector.tensor_tensor(
            prod[:].rearrange("p (c n w) -> p c n w", c=3, n=KW),
            _view(xt[:], [[WP, 3], [1, KW], [1, W]]),
            _view(wt[:], [[0, 3], [W, KW], [1, W]]),
            OP.mult)
        for c in range(3):
            for j in range(KW):
                nc.tensor.matmul(
                    num_wide[:, c * W:(c + 1) * W], ident[:],
                    prod[:, c * WJ + j * W:c * WJ + (j + 1) * W],
                    start=(first_i and j == 0),
                    stop=(last_i and j == 6))

    # Slots 0..6: sub_k + abs_k | DErf_{k-1} | stage-C_{k-2}.  abs_k
    # leads the ACT queue so the abs->s-mm chain never waits on PE's
    # backlog; DErf_{k-1} absorbs the PE wait afterwards.
    for i in range(KH):
        if i == 3:
            emit_A1_m3()
            emit_A2_m3()
        else:
            emit_A1(i)
            emit_A2(i)
        if i >= 1:
            emit_B_m3() if i - 1 == 3 else emit_B(i - 1)
        if i >= 2:
            emit_C_m3() if i - 2 == 3 else emit_C(i - 2)
    emit_B(6)
    emit_C(5)
    emit_C(6)

    rec = finp.tile([H_BAND, W], F32, tag="rec", name="rec")
    # den in [~4e-3, ~60]: approx_fast's ~51 ULP is negligible vs bf16 noise
    nc.vector.reciprocal_approx_fast(rec[:], den_ps[:])
    o = finp.tile([H_BAND, 3 * W], BF16, tag="o", name="o")
    nc.vector.tensor_tensor(
        o[:].rearrange("p (c w) -> p c w", c=3),
        num_wide[:].rearrange("p (c w) -> p c w", c=3),
        _view(rec[:], [[0, 3], [1, W]]), OP.mult)
    nc.sync.dma_start(out_d.ap()[:, :], o[:])


def _build():
    nc = bacc.Bacc("TRN2", target_bir_lowering=False, debug=False)
    g_d = nc.dram_tensor("g", [134, 3 * WP], BF16, kind="ExternalInput")
    x_d = nc.dram_tensor("x", [134, 3 * WP], BF16, kind="ExternalInput")
    e_d = nc.dram_tensor("e", [H_BAND, 16 * W], BF16, kind="ExternalInput")
    gc_d = nc.dram_tensor("gc", [H_BAND, 3 * W], BF16, kind="ExternalInput")
    id_d = nc.dram_tensor("ident", [128, 128], BF16, kind="ExternalInput")
    out_d = nc.dram_tensor("out", [H_BAND, 3 * W], BF16,
                           kind="ExternalOutput")

    with tile.TileContext(nc) as tc:
        with (
            tc.tile_pool(name="const", bufs=1) as constp,
            tc.tile_pool(name="gx", bufs=2) as gxp,
            tc.tile_pool(name="work", bufs=2) as workp,
            tc.tile_pool(name="fin", bufs=1) as finp,
            tc.tile_pool(name="psum", bufs=1, space="PSUM") as psump,
        ):
            _emit(nc, tc, constp, gxp, workp, finp, psump,
                  g_d, x_d, e_d, gc_d, id_d, out_d)

    nc.compile()
    return nc


def _shard_inputs(x, guidance, w0):
    import ml_dtypes
    BF = ml_dtypes.bfloat16
    pad = ((0, 0), (0, 0), (3, 3), (3, 3))
    # (B,3,518,518) -> per-core rows with channels interleaved per row:
    # band[r, c*518 + w]
    xp = np.pad(x, pad, mode="reflect").astype(BF).transpose(0, 2, 1, 3)
    gp = np.pad(guidance, pad, mode="reflect").astype(BF).transpose(0, 2, 1, 3)
    ident = np.eye(128, dtype=BF)

    in_maps = []
    for c in range(8):
        b, band = divmod(c, 4)
        r0 = band * H_BAND
        wslice = w0[b, r0 * W:(r0 + H_BAND) * W]          # (65536, 4, 4)
        e = np.exp(wslice.reshape(H_BAND, W, 4, 4).transpose(0, 2, 3, 1))
        gcore = gp[b, 3 + r0:3 + r0 + H_BAND, :, 3:3 + W]  # (128, 3, 512)
        in_maps.append({
            "g": np.ascontiguousarray(
                gp[b, r0:r0 + H_BAND + 6].reshape(H_BAND + 6, 3 * WP)),
            "x": np.ascontiguousarray(
                xp[b, r0:r0 + H_BAND + 6].reshape(H_BAND + 6, 3 * WP)),
            "e": np.ascontiguousarray(e.reshape(H_BAND, 16 * W)).astype(BF),
            "gc": np.ascontiguousarray(gcore.reshape(H_BAND, 3 * W)),
            "ident": ident,
        })
    return in_maps


def kernel(x, guidance, w0):
    x = np.asarray(x, dtype=np.float32)
    guidance = np.asarray(guidance, dtype=np.float32)
    w0 = np.asarray(w0, dtype=np.float32)
    B, C, H, Wf = x.shape

    if "nc" not in _CACHE:
        _CACHE["nc"] = _build()
    nc = _CACHE["nc"]

    in_maps = _shard_inputs(x, guidance, w0)
    res = run_bass_kernel_spmd(nc, in_maps, core_ids=list(range(8)))

    out = np.empty((B, C, H, Wf), dtype=np.float32)
    for c in range(8):
        b, band = divmod(c, 4)
        r0 = band * H_BAND
        # (128, 3*512) bf16 -> (3, 128, 512) f32
        blk = res.results[c]["out"].astype(np.float32).reshape(
            H_BAND, 3, Wf).transpose(1, 0, 2)
        out[b, :, r0:r0 + H_BAND, :] = blk
    return out


# revision 28
# speedup vs baseline: 1.0133x; 1.0133x over previous
"""Adaptive bilateral filter (nn_AdaptiveFilter) on 8 TRN2 NeuronCores.

Math: out_c(p) = sum_k x_c(p+d_k) * wt_k(p) / sum_k wt_k(p)
with wt_k = softmax_k(w)(p) * exp(-50 * (sum_c |g_c(p+d_k) - g_c(p)|)^2).
Softmax normalization cancels in num/den, so wt_k = E[src(k)] * exp(-50*s^2)
with E = exp(w0) precomputed on HOST (slot-major bf16) and src = reflect
map (7,7)->(4,4).

Sharding: 8 cores = 2 batches x 4 row-bands of 128 rows. Host reflect-pads
to (518,518), converts to bf16 and interleaves channels per row, shipping
each core a (134, 3*518) band (halo included) of g and x, E [128, 16*512]
bf16 slot-major, and the center tile gc [128, 3*512] bf16. No collectives.
Row-interleaved channels make every DMA descriptor a 3108-byte contiguous
run (vs 1036 for planar) and need no kernel-side AP changes.

Engine split per tap-row i (j-packed over 7 column taps, c-packed over 3
channels):
  DVE:    ONE bf16 subtract [128, 3*7*512] (sliding-window in0 vs broadcast
          center in1), wt = col*E (+-512-stride E views), ONE product
          x*wt [128, 3*7*512]
  ACT:    in-place Abs on the subtract output, Derivative_Erf(sqrt(50)*s)
          = 2/sqrt(pi)*exp(-50 s^2) from 2-bank PSUM pairs (the 2/sqrt(pi)
          cancels between num and den)
  PE:     channel-sum of |d| into PSUM (identity matmuls), den/num
          accumulation over the 49 taps
  DMA:    g on the sync queue, x + E chunks on the scalar queue; E_t is
          issued in slot t (not up-front) so it never starves g0/x0.
Emission is software-pipelined: slot k runs sub_k | DErf_{k-1} | abs_k |
wt/prod/den/num_{k-2}. Output is one packed bf16 [128, 3*512] DMA
(host casts up and de-interleaves).
"""
import sys
sys.path.insert(0, "/opt/trn_rl_repo")
import math
import numpy as np

import concourse.bacc as bacc
import concourse.mybir as mybir
import concourse.tile as tile
from concourse.ap import AP
from concourse.bass_utils import run_bass_kernel_spmd

F32 = mybir.dt.float32
U16 = mybir.dt.uint16
BF16 = mybir.dt.bfloat16
AF = mybir.ActivationFunctionType
OP = mybir.AluOpType

KH = KW = 7
H_BAND = 128
W = 512
WP = 518
WJ = KW * W        # 3584
CJ = 3 * WJ        # 10752
SCALE = math.sqrt(50.0)  # Square(sqrt(50)*s) = 50*s^2
PAIRS = ((0, 2), (2, 2), (4, 2), (6, 1))

_CACHE = {}


def _view(ap_obj, dims):
    """AP with the tile's partition dim plus the given free [stride, size]."""
    base = ap_obj.ap
    return AP(tensor=ap_obj.tensor, offset=ap_obj.offset,
              ap=[list(base[0])] + [list(d) for d in dims])


def _emit(nc, tc, constp, gxp, workp, finp, psump, g_d, x_d, e_d, gc_d,
          id_d, out_d):
    ident = constp.tile([128, 128], BF16, tag="ident", name="ident")
    nc.sync.dma_start(ident[:], id_d.ap()[:, :])

    gc = constp.tile([H_BAND, 3 * W], BF16, tag="gc", name="gc")
    nc.sync.dma_start(gc[:], gc_d.ap()[:, :])

    E = [constp.tile([H_BAND, 4 * W], BF16, tag=f"E{t}", name=f"E{t}")
         for t in range(4)]

    den_ps = psump.tile([H_BAND, W], F32, tag="dps", name="dps", bufs=1)
    num_wide = psump.tile([H_BAND, 3 * W], F32, tag="npsw", name="npsw",
                          bufs=1)

    stageA = {}
    stageB = {}

    def emit_A1(i):
        gt = gxp.tile([H_BAND, 3 * WP], BF16, tag="gt", name="gt", bufs=2)
        nc.sync.dma_start(gt[:], g_d.ap()[i:i + H_BAND, :])
        if i < 4:
            nc.sync.dma_start(E[i][:],
                              e_d.ap()[:, i * 4 * W:(i + 1) * 4 * W])
        # u[p, c, j, w] = gt[p, c*518 + j + w] - gc[p, c*512 + w]
        u = workp.tile([H_BAND, CJ], BF16, tag="u", name="u", bufs=2)
        nc.vector.tensor_tensor(
            u[:].rearrange("p (c n w) -> p c n w", c=3, n=KW),
            _view(gt[:], [[WP, 3], [1, KW], [1, W]]),
            _view(gc[:], [[W, 3], [0, KW], [1, W]]),
            OP.subtract)
        stageA[i] = (u, None)

    def emit_A2(i):
        u, _ = stageA[i]
        nc.scalar.activation(u[:], u[:], AF.Abs)

    def emit_B(i):
        u, _ = stageA.pop(i)
        # x_i isn't read until stage C (next slot): issuing its DMA here
        # keeps the early sync queue clear for g/E
        xt = gxp.tile([H_BAND, 3 * WP], BF16, tag="xt", name="xt", bufs=3)
        nc.sync.dma_start(xt[:], x_d.ap()[i:i + H_BAND, :])
        col = workp.tile([H_BAND, WJ], BF16, tag="col", name="col", bufs=2)
        for j in range(KW):
            s_ps = psump.tile([H_BAND, W], F32, tag="sps", name="sps",
                              bufs=4)
            for c in range(3):
                nc.tensor.matmul(
                    s_ps[:], ident[:],
                    u[:, c * WJ + j * W:c * WJ + (j + 1) * W],
                    start=(c == 0), stop=(c == 2))
            nc.scalar.activation(col[:, j * W:(j + 1) * W], s_ps[:],
                                 AF.Derivative_Erf, scale=SCALE)
        stageB[i] = (col, xt)

    TB = 516          # T block stride (515 valid + 1 pad)
    T3C = 3 * TB      # per-channel span of the 3 delta blocks in u3

    def emit_A1_m3():
        gt = gxp.tile([H_BAND, 3 * WP], BF16, tag="gt", name="gt", bufs=2)
        nc.sync.dma_start(gt[:], g_d.ap()[3:3 + H_BAND, :])
        nc.sync.dma_start(E[3][:], e_d.ap()[:, 12 * W:16 * W])
        # u3[p, c, b, a] = g3[p, c*518 + a + (3-b)] - g3[p, c*518 + a]
        u3 = workp.tile([H_BAND, 3 * T3C], BF16, tag="u", name="u", bufs=2)
        nc.vector.tensor_tensor(
            _view(u3[:], [[T3C, 3], [TB, 3], [1, 515]]),
            AP(tensor=gt[:].tensor, offset=gt[:].offset + 3,
               ap=[list(gt[:].ap[0]), [WP, 3], [-1, 3], [1, 515]]),
            _view(gt[:], [[WP, 3], [0, 3], [1, 515]]),
            OP.subtract)
        stageA[3] = (u3, None)

    def emit_A2_m3():
        u3, _ = stageA[3]
        nc.scalar.activation(u3[:], u3[:], AF.Abs)

    def emit_B_m3():
        u3, _ = stageA.pop(3)
        xt = gxp.tile([H_BAND, 3 * WP], BF16, tag="xt", name="xt", bufs=3)
        nc.sync.dma_start(xt[:], x_d.ap()[3:3 + H_BAND, :])
        # T: 3 blocks of 516 (D_3, D_2, D_1), each valid [0, 515)
        colT = workp.tile([H_BAND, T3C], BF16, tag="col", name="col", bufs=2)
        for b in range(3):
            sa = psump.tile([H_BAND, W], F32, tag="sps", name="sps", bufs=4)
            sb = psump.tile([H_BAND, W], F32, tag="sps", name="sps", bufs=4)
            for c in range(3):
                nc.tensor.matmul(sa[:], ident[:],
                                 u3[:, c * T3C + b * TB:c * T3C + b * TB + W],
                                 start=(c == 0), stop=(c == 2))
            for c in range(3):
                nc.tensor.matmul(
                    sb[:, 0:3], ident[:],
                    u3[:, c * T3C + b * TB + W:c * T3C + b * TB + 515],
                    start=(c == 0), stop=(c == 2))
            nc.scalar.activation(colT[:, b * TB:b * TB + W], sa[:],
                                 AF.Derivative_Erf, scale=SCALE)
            nc.scalar.activation(colT[:, b * TB + W:b * TB + 515],
                                 sb[:, 0:3], AF.Derivative_Erf, scale=SCALE)
        stageB[3] = (colT, xt)

    def emit_C_m3():
        colT, xt = stageB.pop(3)
        eb = E[3][:]
        wt = workp.tile([H_BAND, WJ], BF16, tag="wt", name="wt", bufs=2)
        # j=0..2: col_j(w) = T[block 3-j.., w+j] -> offset j*(TB+1)
        nc.vector.tensor_tensor(
            wt[:, 0:3 * W].rearrange("p (n w) -> p n w", n=3),
            AP(tensor=colT[:].tensor, offset=colT[:].offset,
               ap=[list(colT[:].ap[0]), [TB + 1, 3], [1, W]]),
            _view(eb, [[W, 3], [1, W]]), OP.mult)
        # j=3: D_0 == 1, but every other tap carries DErf's 2/sqrt(pi)
        # factor, so scale E by it to keep the ratio consistent
        nc.vector.tensor_scalar(wt[:, 3 * W:4 * W], eb[:, 3 * W:4 * W],
                                2.0 / math.sqrt(math.pi), None, OP.mult)
        # j=4..6: col_j(w) = T[block j-3, w+3] -> offset (6-j)*TB+3
        nc.vector.tensor_tensor(
            wt[:, 4 * W:].rearrange("p (n w) -> p n w", n=3),
            AP(tensor=colT[:].tensor, offset=colT[:].offset + 2 * TB + 3,
               ap=[list(colT[:].ap[0]), [-TB, 3], [1, W]]),
            AP(tensor=eb.tensor, offset=eb.offset + 2 * W,
               ap=[list(eb.ap[0]), [-W, 3], [1, W]]), OP.mult)
        for j in range(KW):
            nc.tensor.matmul(den_ps[:], ident[:], wt[:, j * W:(j + 1) * W],
                             start=False, stop=False)
        prod = workp.tile([H_BAND, CJ], BF16, tag="pr", name="pr", bufs=2)
        nc.vector.tensor_tensor(
            prod[:].rearrange("p (c n w) -> p c n w", c=3, n=KW),
            _view(xt[:], [[WP, 3], [1, KW], [1, W]]),
            _view(wt[:], [[0, 3], [W, KW], [1, W]]),
            OP.mult)
        for c in range(3):
            for j in range(KW):
                nc.tensor.matmul(
                    num_wide[:, c * W:(c + 1) * W], ident[:],
                    prod[:, c * WJ + j * W:c * WJ + (j + 1) * W],
                    start=False, stop=False)

    def emit_C(i):
        col, xt = stageB.pop(i)
        ri = min(i, 6 - i)
        first_i, last_i = (i == 0), (i == 6)
        eb = E[ri][:]
        # wt = col * E(src tap): j in 0..3 reads E[ri] slots 0..3 (+W step),
        # j in 4..6 reads slots 2..0 (-W step)
        wt = workp.tile([H_BAND, WJ], BF16, tag="wt", name="wt", bufs=2)
        nc.vector.tensor_tensor(
            wt[:, 0:4 * W].rearrange("p (n w) -> p n w", n=4),
            col[:, 0:4 * W].rearrange("p (n w) -> p n w", n=4),
            _view(eb, [[W, 4], [1, W]]), OP.mult)
        nc.vector.tensor_tensor(
            wt[:, 4 * W:].rearrange("p (n w) -> p n w", n=3),
            col[:, 4 * W:].rearrange("p (n w) -> p n w", n=3),
            AP(tensor=eb.tensor, offset=eb.offset + 2 * W,
               ap=[list(eb.ap[0]), [-W, 3], [1, W]]), OP.mult)
        for j in range(KW):
            nc.tensor.matmul(den_ps[:], ident[:], wt[:, j * W:(j + 1) * W],
                             start=(first_i and j == 0),
                             stop=(last_i and j == 6))
        # prod[p, c, j, w] = xt[p, c*518 + j + w] * wt[p, j*512 + w]
        prod = workp.tile([H_BAND, CJ], BF16, tag="pr", name="pr", bufs=2)
        nc.vector.tensor_tensor(
            prod[:].rearrange("p (c n w) -> p c n w", c=3, n=KW),
            _view(xt[:], [[WP, 3], [1, KW], [1, W]]),
            _view(wt[:], [[0, 3], [W, KW], [1, W]]),
            OP.mult)
        for c in range(3):
            for j in range(KW):
                nc.tensor.matmul(
                    num_wide[:, c * W:(c + 1) * W], ident[:],
                    prod[:, c * WJ + j * W:c * WJ + (j + 1) * W],
                    start=(first_i and j == 0),
                    stop=(last_i and j == 6))

    # Slots 0..6: sub_k + abs_k | DErf_{k-1} | stage-C_{k-2}.  abs_k
    # leads the ACT queue so the abs->s-mm chain never waits on PE's
    # backlog; DErf_{k-1} absorbs the PE wait afterwards.
    for i in range(KH):
        if i == 3:
            emit_A1_m3()
            emit_A2_m3()
        else:
            emit_A1(i)
            emit_A2(i)
        if i >= 1:
            emit_B_m3() if i - 1 == 3 else emit_B(i - 1)
        if i >= 2:
            emit_C_m3() if i - 2 == 3 else emit_C(i - 2)
    emit_B(6)
    emit_C(5)
    emit_C(6)

    rec = finp.tile([H_BAND, W], F32, tag="rec", name="rec")
    # den in [~4e-3, ~60]: approx_fast's ~51 ULP is negligible vs bf16 noise
    nc.vector.reciprocal_approx_fast(rec[:], den_ps[:])
    o = finp.tile([H_BAND, 3 * W], BF16, tag="o", name="o")
    nc.vector.tensor_tensor(
        o[:].rearrange("p (c w) -> p c w", c=3),
        num_wide[:].rearrange("p (c w) -> p c w", c=3),
        _view(rec[:], [[0, 3], [1, W]]), OP.mult)
    nc.sync.dma_start(out_d.ap()[:, :], o[:])


def _build():
    nc = bacc.Bacc("TRN2", target_bir_lowering=False, debug=False)
    g_d = nc.dram_tensor("g", [134, 3 * WP], BF16, kind="ExternalInput")
    x_d = nc.dram_tensor("x", [134, 3 * WP], BF16, kind="ExternalInput")
    e_d = nc.dram_tensor("e", [H_BAND, 16 * W], BF16, kind="ExternalInput")
    gc_d = nc.dram_tensor("gc", [H_BAND, 3 * W], BF16, kind="ExternalInput")
    id_d = nc.dram_tensor("ident", [128, 128], BF16, kind="ExternalInput")
    out_d = nc.dram_tensor("out", [H_BAND, 3 * W], BF16,
                           kind="ExternalOutput")

    with tile.TileContext(nc) as tc:
        with (
            tc.tile_pool(name="const", bufs=1) as constp,
            tc.tile_pool(name="gx", bufs=2) as gxp,
            tc.tile_pool(name="work", bufs=2) as workp,
            tc.tile_pool(name="fin", bufs=1) as finp,
            tc.tile_pool(name="psum", bufs=1, space="PSUM") as psump,
        ):
            _emit(nc, tc, constp, gxp, workp, finp, psump,
                  g_d, x_d, e_d, gc_d, id_d, out_d)

    nc.compile()
    return nc


def _shard_inputs(x, guidance, w0):
    import ml_dtypes
    BF = ml_dtypes.bfloat16
    pad = ((0, 0), (0, 0), (3, 3), (3, 3))
    # (B,3,518,518) -> per-core rows with channels interleaved per row:
    # band[r, c*518 + w]
    xp = np.pad(x, pad, mode="reflect").astype(BF).transpose(0, 2, 1, 3)
    gp = np.pad(guidance, pad, mode="reflect").astype(BF).transpose(0, 2, 1, 3)
    ident = np.eye(128, dtype=BF)

    in_maps = []
    for c in range(8):
        b, band = divmod(c, 4)
        r0 = band * H_BAND
        wslice = w0[b, r0 * W:(r0 + H_BAND) * W]          # (65536, 4, 4)
        e = np.exp(wslice.reshape(H_BAND, W, 4, 4).transpose(0, 2, 3, 1))
        gcore = gp[b, 3 + r0:3 + r0 + H_BAND, :, 3:3 + W]  # (128, 3, 512)
        in_maps.append({
            "g": np.ascontiguousarray(
                gp[b, r0:r0 + H_BAND + 6].reshape(H_BAND + 6, 3 * WP)),
            "x": np.ascontiguousarray(
                xp[b, r0:r0 + H_BAND + 6].reshape(H_BAND + 6, 3 * WP)),
            "e": np.ascontiguousarray(e.reshape(H_BAND, 16 * W)).astype(BF),
            "gc": np.ascontiguousarray(gcore.reshape(H_BAND, 3 * W)),
            "ident": ident,
        })
    return in_maps


def kernel(x, guidance, w0):
    x = np.asarray(x, dtype=np.float32)
    guidance = np.asarray(guidance, dtype=np.float32)
    w0 = np.asarray(w0, dtype=np.float32)
    B, C, H, Wf = x.shape

    if "nc" not in _CACHE:
        _CACHE["nc"] = _build()
    nc = _CACHE["nc"]

    in_maps = _shard_inputs(x, guidance, w0)
    res = run_bass_kernel_spmd(nc, in_maps, core_ids=list(range(8)))

    out = np.empty((B, C, H, Wf), dtype=np.float32)
    for c in range(8):
        b, band = divmod(c, 4)
        r0 = band * H_BAND
        # (128, 3*512) bf16 -> (3, 128, 512) f32
        blk = res.results[c]["out"].astype(np.float32).reshape(
            H_BAND, 3, Wf).transpose(1, 0, 2)
        out[b, :, r0:r0 + H_BAND, :] = blk
    return out


# revision 30
# speedup vs baseline: 1.0297x; 1.0162x over previous
"""Adaptive bilateral filter (nn_AdaptiveFilter) on 8 TRN2 NeuronCores.

Math: out_c(p) = sum_k x_c(p+d_k) * wt_k(p) / sum_k wt_k(p)
with wt_k = softmax_k(w)(p) * exp(-50 * (sum_c |g_c(p+d_k) - g_c(p)|)^2).
Softmax normalization cancels in num/den, so wt_k = E[src(k)] * exp(-50*s^2)
with E = exp(w0) precomputed on HOST (slot-major bf16) and src = reflect
map (7,7)->(4,4).

Sharding: 8 cores = 2 batches x 4 row-bands of 128 rows. Host reflect-pads
to (518,518), converts to bf16 and interleaves channels per row, shipping
each core a (134, 3*518) band (halo included) of g and x, E [128, 16*512]
bf16 slot-major, and the center tile gc [128, 3*512] bf16. No collectives.
Row-interleaved channels make every DMA descriptor a 3108-byte contiguous
run (vs 1036 for planar) and need no kernel-side AP changes.

Engine split per tap-row i (j-packed over 7 column taps, c-packed over 3
channels):
  DVE:    ONE bf16 subtract [128, 3*7*512] (sliding-window in0 vs broadcast
          center in1), wt = col*E (+-512-stride E views), ONE product
          x*wt [128, 3*7*512]
  ACT:    in-place Abs on the subtract output, Derivative_Erf(sqrt(50)*s)
          = 2/sqrt(pi)*exp(-50 s^2) from 2-bank PSUM pairs (the 2/sqrt(pi)
          cancels between num and den)
  PE:     channel-sum of |d| into PSUM (identity matmuls), den/num
          accumulation over the 49 taps
  DMA:    g on the sync queue, x + E chunks on the scalar queue; E_t is
          issued in slot t (not up-front) so it never starves g0/x0.
Emission is software-pipelined: slot k runs sub_k | DErf_{k-1} | abs_k |
wt/prod/den/num_{k-2}. Output is one packed bf16 [128, 3*512] DMA
(host casts up and de-interleaves).
"""
import sys
sys.path.insert(0, "/opt/trn_rl_repo")
import math
import numpy as np

import concourse.bacc as bacc
import concourse.mybir as mybir
import concourse.tile as tile
from concourse.ap import AP
from concourse.bass_utils import run_bass_kernel_spmd

F32 = mybir.dt.float32
U16 = mybir.dt.uint16
BF16 = mybir.dt.bfloat16
AF = mybir.ActivationFunctionType
OP = mybir.AluOpType

KH = KW = 7
H_BAND = 128
W = 512
WP = 518
WJ = KW * W        # 3584
CJ = 3 * WJ        # 10752
SCALE = math.sqrt(50.0)  # Square(sqrt(50)*s) = 50*s^2
PAIRS = ((0, 2), (2, 2), (4, 2), (6, 1))

_CACHE = {}


def _view(ap_obj, dims):
    """AP with the tile's partition dim plus the given free [stride, size]."""
    base = ap_obj.ap
    return AP(tensor=ap_obj.tensor, offset=ap_obj.offset,
              ap=[list(base[0])] + [list(d) for d in dims])


def _emit(nc, tc, constp, gxp, workp, finp, psump, g_d, x_d, e_d, gc_d,
          id_d, out_d):
    ident = constp.tile([128, 128], BF16, tag="ident", name="ident")
    nc.sync.dma_start(ident[:], id_d.ap()[:, :])

    gc = constp.tile([H_BAND, 3 * W], BF16, tag="gc", name="gc")
    nc.sync.dma_start(gc[:], gc_d.ap()[:, :])

    E = [constp.tile([H_BAND, 4 * W], BF16, tag=f"E{t}", name=f"E{t}")
         for t in range(4)]

    den_ps = psump.tile([H_BAND, W], F32, tag="dps", name="dps", bufs=1)
    num_wide = psump.tile([H_BAND, 3 * W], F32, tag="npsw", name="npsw",
                          bufs=1)

    stageA = {}
    stageB = {}

    def emit_A1(i):
        gt = gxp.tile([H_BAND, 3 * WP], BF16, tag="gt", name="gt", bufs=2)
        nc.sync.dma_start(gt[:], g_d.ap()[i:i + H_BAND, :])
        if i < 4:
            nc.sync.dma_start(E[i][:],
                              e_d.ap()[:, i * 4 * W:(i + 1) * 4 * W])
        # u[p, c, j, w] = gt[p, c*518 + j + w] - gc[p, c*512 + w]
        u = workp.tile([H_BAND, CJ], BF16, tag="u", name="u", bufs=2)
        nc.vector.tensor_tensor(
            u[:].rearrange("p (c n w) -> p c n w", c=3, n=KW),
            _view(gt[:], [[WP, 3], [1, KW], [1, W]]),
            _view(gc[:], [[W, 3], [0, KW], [1, W]]),
            OP.subtract)
        stageA[i] = (u, None)

    def emit_A2(i):
        u, _ = stageA[i]
        nc.scalar.activation(u[:], u[:], AF.Abs)

    def emit_B(i):
        u, _ = stageA.pop(i)
        # x_i isn't read until stage C (next slot): issuing its DMA here
        # keeps the early sync queue clear for g/E
        xt = gxp.tile([H_BAND, 3 * WP], BF16, tag="xt", name="xt", bufs=3)
        nc.sync.dma_start(xt[:], x_d.ap()[i:i + H_BAND, :])
        col = workp.tile([H_BAND, WJ], BF16, tag="col", name="col", bufs=2)
        for j in range(KW):
            s_ps = psump.tile([H_BAND, W], F32, tag="sps", name="sps",
                              bufs=4)
            for c in range(3):
                nc.tensor.matmul(
                    s_ps[:], ident[:],
                    u[:, c * WJ + j * W:c * WJ + (j + 1) * W],
                    start=(c == 0), stop=(c == 2))
            nc.scalar.activation(col[:, j * W:(j + 1) * W], s_ps[:],
                                 AF.Derivative_Erf, scale=SCALE)
        stageB[i] = (col, xt)

    def emit_C(i):
        col, xt = stageB.pop(i)
        ri = min(i, 6 - i)
        first_i, last_i = (i == 0), (i == 6)
        eb = E[ri][:]
        # wt = col * E(src tap): j in 0..3 reads E[ri] slots 0..3 (+W step),
        # j in 4..6 reads slots 2..0 (-W step)
        wt = workp.tile([H_BAND, WJ], BF16, tag="wt", name="wt", bufs=2)
        nc.vector.tensor_tensor(
            wt[:, 0:4 * W].rearrange("p (n w) -> p n w", n=4),
            col[:, 0:4 * W].rearrange("p (n w) -> p n w", n=4),
            _view(eb, [[W, 4], [1, W]]), OP.mult)
        nc.vector.tensor_tensor(
            wt[:, 4 * W:].rearrange("p (n w) -> p n w", n=3),
            col[:, 4 * W:].rearrange("p (n w) -> p n w", n=3),
            AP(tensor=eb.tensor, offset=eb.offset + 2 * W,
               ap=[list(eb.ap[0]), [-W, 3], [1, W]]), OP.mult)
        for j in range(KW):
            nc.tensor.matmul(den_ps[:], ident[:], wt[:, j * W:(j + 1) * W],
                             start=(first_i and j == 0),
                             stop=(last_i and j == 6))
        # prod[p, c, j, w] = xt[p, c*518 + j + w] * wt[p, j*512 + w]
        prod = workp.tile([H_BAND, CJ], BF16, tag="pr", name="pr", bufs=2)
        nc.vector.tensor_tensor(
            prod[:].rearrange("p (c n w) -> p c n w", c=3, n=KW),
            _view(xt[:], [[WP, 3], [1, KW], [1, W]]),
            _view(wt[:], [[0, 3], [W, KW], [1, W]]),
            OP.mult)
        for c in range(3):
            for j in range(KW):
                nc.tensor.matmul(
                    num_wide[:, c * W:(c + 1) * W], ident[:],
                    prod[:, c * WJ + j * W:c * WJ + (j + 1) * W],
                    start=(first_i and j == 0),
                    stop=(last_i and j == 6))

    # Slots 0..6: sub_k + abs_k | DErf_{k-1} | stage-C_{k-2}.  abs_k
    # leads the ACT queue so the abs->s-mm chain never waits on PE's
    # backlog; DErf_{k-1} absorbs the PE wait afterwards.
    def emit_A1_split0():
        gt = gxp.tile([H_BAND, 3 * WP], BF16, tag="gt", name="gt", bufs=2)
        nc.sync.dma_start(gt[:], g_d.ap()[0:H_BAND, :])
        nc.sync.dma_start(E[0][:], e_d.ap()[:, 0:4 * W])
        u = workp.tile([H_BAND, CJ], BF16, tag="u", name="u", bufs=2)
        for j0, nj in ((0, 4), (4, 3)):
            nc.vector.tensor_tensor(
                AP(tensor=u[:].tensor, offset=u[:].offset + j0 * W,
                   ap=[list(u[:].ap[0]), [WJ, 3], [W, nj], [1, W]]),
                AP(tensor=gt[:].tensor, offset=gt[:].offset + j0,
                   ap=[list(gt[:].ap[0]), [WP, 3], [1, nj], [1, W]]),
                AP(tensor=gc[:].tensor, offset=gc[:].offset,
                   ap=[list(gc[:].ap[0]), [W, 3], [0, nj], [1, W]]),
                OP.subtract)
            v = AP(tensor=u[:].tensor, offset=u[:].offset + j0 * W,
                   ap=[list(u[:].ap[0]), [WJ, 3], [1, nj * W]])
            nc.scalar.activation(v, v, AF.Abs)
        stageA[0] = (u, None)

    for i in range(KH):
        if i == 0:
            emit_A1_split0()
        else:
            emit_A1(i)
            emit_A2(i)
        if i >= 1:
            emit_B(i - 1)
        if i >= 2:
            emit_C(i - 2)
    emit_B(6)
    emit_C(5)
    emit_C(6)

    rec = finp.tile([H_BAND, W], F32, tag="rec", name="rec")
    # den in [~4e-3, ~60]: approx_fast's ~51 ULP is negligible vs bf16 noise
    nc.vector.reciprocal_approx_fast(rec[:], den_ps[:])
    o = finp.tile([H_BAND, 3 * W], BF16, tag="o", name="o")
    nc.vector.tensor_tensor(
        o[:].rearrange("p (c w) -> p c w", c=3),
        num_wide[:].rearrange("p (c w) -> p c w", c=3),
        _view(rec[:], [[0, 3], [1, W]]), OP.mult)
    nc.sync.dma_start(out_d.ap()[:, :], o[:])


def _build():
    nc = bacc.Bacc("TRN2", target_bir_lowering=False, debug=False)
    g_d = nc.dram_tensor("g", [134, 3 * WP], BF16, kind="ExternalInput")
    x_d = nc.dram_tensor("x", [134, 3 * WP], BF16, kind="ExternalInput")
    e_d = nc.dram_tensor("e", [H_BAND, 16 * W], BF16, kind="ExternalInput")
    gc_d = nc.dram_tensor("gc", [H_BAND, 3 * W], BF16, kind="ExternalInput")
    id_d = nc.dram_tensor("ident", [128, 128], BF16, kind="ExternalInput")
    out_d = nc.dram_tensor("out", [H_BAND, 3 * W], BF16,
                           kind="ExternalOutput")

    with tile.TileContext(nc) as tc:
        with (
            tc.tile_pool(name="const", bufs=1) as constp,
            tc.tile_pool(name="gx", bufs=2) as gxp,
            tc.tile_pool(name="work", bufs=2) as workp,
            tc.tile_pool(name="fin", bufs=1) as finp,
            tc.tile_pool(name="psum", bufs=1, space="PSUM") as psump,
        ):
            _emit(nc, tc, constp, gxp, workp, finp, psump,
                  g_d, x_d, e_d, gc_d, id_d, out_d)

    nc.compile()
    return nc


def _shard_inputs(x, guidance, w0):
    import ml_dtypes
    BF = ml_dtypes.bfloat16
    pad = ((0, 0), (0, 0), (3, 3), (3, 3))
    # (B,3,518,518) -> per-core rows with channels interleaved per row:
    # band[r, c*518 + w]
    xp = np.pad(x, pad, mode="reflect").astype(BF).transpose(0, 2, 1, 3)
    gp = np.pad(guidance, pad, mode="reflect").astype(BF).transpose(0, 2, 1, 3)
    ident = np.eye(128, dtype=BF)

    in_maps = []
    for c in range(8):
        b, band = divmod(c, 4)
        r0 = band * H_BAND
        wslice = w0[b, r0 * W:(r0 + H_BAND) * W]          # (65536, 4, 4)
        e = np.exp(wslice.reshape(H_BAND, W, 4, 4).transpose(0, 2, 3, 1))
        gcore = gp[b, 3 + r0:3 + r0 + H_BAND, :, 3:3 + W]  # (128, 3, 512)
        in_maps.append({
            "g": np.ascontiguousarray(
                gp[b, r0:r0 + H_BAND + 6].reshape(H_BAND + 6, 3 * WP)),
            "x": np.ascontiguousarray(
                xp[b, r0:r0 + H_BAND + 6].reshape(H_BAND + 6, 3 * WP)),
            "e": np.ascontiguousarray(e.reshape(H_BAND, 16 * W)).astype(BF),
            "gc": np.ascontiguousarray(gcore.reshape(H_BAND, 3 * W)),
            "ident": ident,
        })
    return in_maps


def kernel(x, guidance, w0):
    x = np.asarray(x, dtype=np.float32)
    guidance = np.asarray(guidance, dtype=np.float32)
    w0 = np.asarray(w0, dtype=np.float32)
    B, C, H, Wf = x.shape

    if "nc" not in _CACHE:
        _CACHE["nc"] = _build()
    nc = _CACHE["nc"]

    in_maps = _shard_inputs(x, guidance, w0)
    res = run_bass_kernel_spmd(nc, in_maps, core_ids=list(range(8)))

    out = np.empty((B, C, H, Wf), dtype=np.float32)
    for c in range(8):
        b, band = divmod(c, 4)
        r0 = band * H_BAND
        # (128, 3*512) bf16 -> (3, 128, 512) f32
        blk = res.results[c]["out"].astype(np.float32).reshape(
            H_BAND, 3, Wf).transpose(1, 0, 2)
        out[b, :, r0:r0 + H_BAND, :] = blk
    return out
